# revision 11
# baseline (speedup 1.0000x reference)
"""Trainium2 Bass kernel for nn_MultiHeadAttention_50861002719805.

Full inputs in, full output out. Sharding: 8 cores = 4 batches x 2 head-groups
(tensor-parallel over heads, data-parallel over batch). Each core computes
attention for its batch + 8 heads, projects through its W_out rows, and an
AllReduce over core pairs {2b, 2b+1} produces the final output for batch b.

Per-core algorithm (all in transposed "head-dim on partitions" layout):
  Q^T = (Wq/32)^T x^T        [64,T] per head   (C**-0.5 folded into Wq)
  K^T = Wk^T x^T             [64,T]
  V   = x Wv                 [T,64]
  S^T[s,t] = K^T[:,s].Q^T[:,t]  computed per [128s x 512t] tile, fp32r
  E = exp(S) (no max-shift needed: |S|<~1.5), masked entries := 1.0
    (faithful to the reference bug: masked scores = 1e-9, exp(1e-9)==1.0f)
  Fully-masked s-tiles (s0 >= t0+512) are skipped; their contribution is the
  rank-1 suffix sum_{s>=t0+512} v[s] (+count into Z), added as a K=1 matmul.
  O^T_aug[65,512] = sum_s v_aug[s,:].E[s,t], v_aug = [v | 1] so row 64 = Z.
  O^T_norm = O^T * (1/Z) broadcast, stored bf16, pair-packed [128,4,T].
  out[t,:] = sum_j O^T_norm[:,j,t]^T @ W_out rows  (bf16), AllReduce pairs.
"""
import numpy as np
import ml_dtypes

import concourse.bacc as bacc
import concourse.mybir as mybir
import concourse.tile as tile
from concourse.bass_utils import run_bass_kernel_spmd

F32 = mybir.dt.float32
F32R = mybir.dt.float32r
BF16 = mybir.dt.bfloat16
U8 = mybir.dt.uint8

B, T, D = 4, 2048, 1024
H, HS = 16, 64          # global heads, head size
HL = 8                  # heads per core
TCH, SCH = 512, 128     # t-chunk (psum free dim), s-chunk (partition tile)
NTC, NSC = T // TCH, T // SCH   # 4, 16
NDC = D // 128          # 8 contraction chunks
NP = 4                  # head pairs per core
ADD = mybir.AluOpType.add
MULT = mybir.AluOpType.mult


def build(reps=1, collective=True):
    nc = bacc.Bacc("TRN2", target_bir_lowering=False, debug=False, num_devices=8)

    xT = nc.declare_dram_parameter("xT", [D, T], F32R, isOutput=False)
    wq = nc.declare_dram_parameter("wq", [D, HL * HS], F32R, isOutput=False)
    wk = nc.declare_dram_parameter("wk", [D, HL * HS], F32R, isOutput=False)
    wv = nc.declare_dram_parameter("wv", [D, HL * HS], F32R, isOutput=False)
    wo = nc.declare_dram_parameter("wo", [HL * HS, D], BF16, isOutput=False)
    mask = nc.declare_dram_parameter("mask", [4, SCH, TCH], U8, isOutput=False)
    out = nc.declare_dram_parameter("out", [T, D], F32, isOutput=True)

    with tile.TileContext(nc) as tc:
      for rep in range(reps):
        with (
            tc.tile_pool(name=f"const{rep}", bufs=1) as cpool,
            tc.tile_pool(name=f"wpool{rep}", bufs=1) as wpool,
            tc.tile_pool(name=f"vstp{rep}", bufs=1) as vstp,
            tc.tile_pool(name=f"ostp{rep}", bufs=1) as ostp,
            tc.tile_pool(name=f"xp{rep}", bufs=2) as xp,
            tc.tile_pool(name=f"qkt{rep}", bufs=2) as qkt,
            tc.tile_pool(name=f"ep{rep}", bufs=2) as ep,
            tc.tile_pool(name=f"small{rep}", bufs=2) as sp,
            tc.tile_pool(name=f"outp{rep}", bufs=2) as outp,
            tc.tile_pool(name=f"dram{rep}", bufs=1, space="DRAM") as dp,
        ):
            partial = dp.tile([T, D], F32)
            red = dp.tile([T, D], F32)
            # ---- constants ----
            ones_col_bf = cpool.tile([128, 1], BF16)        # chunk-sum lhsT
            ones_t_bf = cpool.tile([128, TCH], BF16)        # masked-fill data
            ones_f = cpool.tile([1, TCH], F32)
            ones_r = cpool.tile([1, TCH], F32R)             # rank-1 rhs
            nc.vector.memset(ones_col_bf[:], 1.0)
            nc.vector.memset(ones_t_bf[:], 1.0)
            nc.vector.memset(ones_f[:], 1.0)
            nc.vector.tensor_copy(ones_r[:], ones_f[:])

            mask_sb = cpool.tile([SCH, 4, TCH], U8)
            for k in range(4):
                nc.sync.dma_start(mask_sb[:, k, :], mask[k, :, :])

            # ---- weights ----
            wq_sb = wpool.tile([128, NDC, HL * HS], F32R)
            wk_sb = wpool.tile([128, NDC, HL * HS], F32R)
            wv_sb = wpool.tile([128, NDC, HL * HS], F32R)
            wo_sb = wpool.tile([128, NP, D], BF16)
            for dc in range(NDC):
                nc.sync.dma_start(wq_sb[:, dc, :], wq[dc * 128:(dc + 1) * 128, :])
                nc.sync.dma_start(wk_sb[:, dc, :], wk[dc * 128:(dc + 1) * 128, :])
                nc.sync.dma_start(wv_sb[:, dc, :], wv[dc * 128:(dc + 1) * 128, :])
            for j in range(NP):
                nc.sync.dma_start(wo_sb[:, j, :], wo[j * 128:(j + 1) * 128, :])

            # ---- V phase: V_st[p, sc, h, 0:64]=v, col 64 = 1.0 (ones for Z) ----
            V_st = vstp.tile([SCH, NSC, HL, HS + 1], BF16)
            nc.vector.memset(V_st[:], 1.0)
            with tc.tile_pool(name=f"vps{rep}", bufs=1, space="PSUM") as vps:
                for sub in range(2):
                    pv = [vps.tile([SCH, HL, HS], F32, tag=f"v{i}", name=f"pv{rep}_{sub}_{i}") for i in range(8)]
                    for dc in range(NDC):
                        xt = xp.tile([128, T], F32R, tag="xqk", name=f"xtv{rep}_{sub}_{dc}")
                        nc.sync.dma_start(
                            xt[:, 0:T // 2], xT[dc * 128:(dc + 1) * 128,
                                                sub * 1024:(sub + 1) * 1024])
                        for i in range(8):
                            nc.tensor.matmul(
                                pv[i][:], xt[:, i * 128:(i + 1) * 128],
                                wv_sb[:, dc, :],
                                start=(dc == 0), stop=(dc == NDC - 1))
                    for i in range(8):
                        sc = sub * 8 + i
                        nc.vector.tensor_copy(V_st[:, sc, :, 1:HS + 1], pv[i][:])

            # ---- suffix sums: Vsuf_r[0, tc, half, h*65:(h+1)*65] ----
            # Vsuf[tc] = sum over chunks c >= 4(tc+1) of column-sums of V_st
            # (incl. ones col -> masked count). Two 260-wide halves (4 heads each).
            HWID = 4 * (HS + 1)  # 260
            vsuf_r = cpool.tile([1, 3, 2, HWID], F32R)
            with tc.tile_pool(name=f"sfps{rep}", bufs=1, space="PSUM") as sfps:
                for tcb in range(3):
                    for half in range(2):
                        psf = sfps.tile([1, HWID], F32, tag=f"sf{tcb}{half}")
                        lo = 4 * (tcb + 1)
                        for c in range(lo, NSC):
                            nc.tensor.matmul(
                                psf[:], ones_col_bf[:],
                                V_st[:, c, half * 4:(half + 1) * 4, :],
                                start=(c == lo), stop=(c == NSC - 1))
                        nc.vector.tensor_copy(vsuf_r[0:1, tcb, half, :], psf[:])

            # ---- O^T storage: pair-packed [128, pair, T] bf16 ----
            O_st = ostp.tile([128, NP, T], BF16)

            for j in range(NP):
                # -- QK phase for pair j --
                QT = qkt.tile([128, NTC, TCH], F32R, tag="qt")
                KT = qkt.tile([128, NTC, TCH], F32R, tag="kt")
                with tc.tile_pool(name=f"qkps{rep}_{j}", bufs=1, space="PSUM") as qkps:
                    pq = [qkps.tile([128, TCH], F32, tag=f"q{i}", name=f"pq{rep}_{j}_{i}") for i in range(NTC)]
                    pk = [qkps.tile([128, TCH], F32, tag=f"k{i}", name=f"pk{rep}_{j}_{i}") for i in range(NTC)]
                    for dc in range(NDC):
                        xt = xp.tile([128, T], F32R, tag="xqk")
                        nc.sync.dma_start(xt[:], xT[dc * 128:(dc + 1) * 128, :])
                        for tcb in range(NTC):
                            nc.tensor.matmul(
                                pq[tcb][:],
                                wq_sb[:, dc, j * 128:(j + 1) * 128],
                                xt[:, tcb * TCH:(tcb + 1) * TCH],
                                start=(dc == 0), stop=(dc == NDC - 1))
                            nc.tensor.matmul(
                                pk[tcb][:],
                                wk_sb[:, dc, j * 128:(j + 1) * 128],
                                xt[:, tcb * TCH:(tcb + 1) * TCH],
                                start=(dc == 0), stop=(dc == NDC - 1))
                    for tcb in range(NTC):
                        nc.vector.tensor_copy(QT[:, tcb, :], pq[tcb][:])
                        nc.vector.tensor_copy(KT[:, tcb, :], pk[tcb][:])

                # -- attention for heads (2j, 2j+1) --
                with (
                    tc.tile_pool(name=f"sps{rep}_{j}", bufs=4, space="PSUM") as spsum,
                    tc.tile_pool(name=f"ops{rep}_{j}", bufs=4, space="PSUM") as opsum,
                ):
                    for tcb in range(NTC):
                        nv = 4 * (tcb + 1)   # valid s-chunks
                        E = [ep.tile([SCH, NSC, TCH], BF16, tag="E", name=f"E{rep}_{j}_{tcb}_{ee}") for ee in range(2)]
                        po = [opsum.tile([HS + 1, TCH], F32, tag="po", name=f"po{rep}_{j}_{tcb}_{ee}") for ee in range(2)]
                        for c in range(nv):
                            ps = [None, None]
                            for e in range(2):
                                ps[e] = spsum.tile([SCH, TCH], F32, tag="ps", name=f"ps{rep}_{j}_{tcb}_{c}_{e}")
                                nc.tensor.matmul(
                                    ps[e][:],
                                    KT[64 * e:64 * e + 64, c // 4,
                                       (c % 4) * SCH:(c % 4 + 1) * SCH],
                                    QT[64 * e:64 * e + 64, tcb, :],
                                    start=True, stop=True)
                            for e in range(2):
                                nc.scalar.activation(
                                    E[e][:, c, :], ps[e][:],
                                    mybir.ActivationFunctionType.Exp)
                                if c >= 4 * tcb:
                                    nc.vector.copy_predicated(
                                        E[e][:, c, :],
                                        mask_sb[:, c - 4 * tcb, :],
                                        ones_t_bf[:])
                            for e in range(2):
                                h = 2 * j + e
                                nc.tensor.matmul(
                                    po[e][:],
                                    V_st[:, c, h, :],
                                    E[e][:, c, :],
                                    start=(c == 0), stop=(c == nv - 1 and tcb == 3),
                                    skip_group_check=True)
                        if tcb < 3:
                            for e in range(2):
                                h = 2 * j + e
                                nc.tensor.matmul(
                                    po[e][:],
                                    vsuf_r[0:1, tcb, j // 2,
                                           (h % 4) * (HS + 1):(h % 4 + 1) * (HS + 1)],
                                    ones_r[:],
                                    start=False, stop=True,
                                    skip_group_check=True)
                        # -- normalize: O^T / Z (Z is psum row 0) --
                        for e in range(2):
                            rp0 = sp.tile([1, TCH], F32, tag="rp0")
                            nc.vector.reciprocal(rp0[:], po[e][0:1, :])
                            rbc = sp.tile([HS + 1, TCH], F32, tag="rbc")
                            nc.gpsimd.partition_broadcast(
                                rbc[:], rp0[:], channels=HS + 1)
                            og = sp.tile([HS + 1, TCH], BF16, tag="og")
                            nc.vector.tensor_tensor(
                                og[:], po[e][:], rbc[:], MULT)
                            nc.sync.dma_start(
                                O_st[64 * e:64 * e + 64, j,
                                     tcb * TCH:(tcb + 1) * TCH],
                                og[1:HS + 1, :])

            # ---- output projection ----
            with tc.tile_pool(name=f"pps{rep}", bufs=4, space="PSUM") as pps:
                for tt in range(T // 128):
                    ob = outp.tile([128, D], F32, tag="ob")
                    for co in range(2):
                        pp = pps.tile([128, TCH], F32, tag="pp")
                        for j in range(NP):
                            nc.tensor.matmul(
                                pp[:],
                                O_st[:, j, tt * 128:(tt + 1) * 128],
                                wo_sb[:, j, co * TCH:(co + 1) * TCH],
                                start=(j == 0), stop=(j == NP - 1))
                        nc.vector.tensor_copy(ob[:, co * TCH:(co + 1) * TCH], pp[:])
                    nc.sync.dma_start(partial[tt * 128:(tt + 1) * 128, :], ob[:])

            # ---- AllReduce over pairs + writeback ----
            if collective:
                nc.gpsimd.collective_compute(
                    "AllReduce", ADD,
                    replica_groups=[[0, 1], [2, 3], [4, 5], [6, 7]],
                    ins=[partial[:]],
                    outs=[red[:]],
                )
                nc.sync.dma_start(out[:], red[:])
            else:
                nc.sync.dma_start(out[:], partial[:])

    nc.compile()
    return nc


def make_mask():
    # mask[k][p, f] = 1 where masked: s > t  <=>  p + 128k > f
    p = np.arange(SCH)[:, None]
    f = np.arange(TCH)[None, :]
    return np.stack([(p + 128 * k > f) for k in range(4)]).astype(np.uint8)


def make_in_maps(x, W_qkv, W_out):
    x = np.asarray(x, dtype=np.float32)
    W_qkv = np.asarray(W_qkv, dtype=np.float32)
    W_out = np.asarray(W_out, dtype=np.float32)
    mask = make_mask()
    in_maps = []
    for c in range(8):
        b, hg = c // 2, c % 2
        heads = slice(hg * HL, (hg + 1) * HL)
        # [h, d, f] -> [d, h, f] -> [d, h*f]
        wq_h = W_qkv[heads, :, 0:HS].transpose(1, 0, 2).reshape(D, HL * HS) * (1.0 / 32.0)
        wk_h = W_qkv[heads, :, HS:2 * HS].transpose(1, 0, 2).reshape(D, HL * HS)
        wv_h = W_qkv[heads, :, 2 * HS:3 * HS].transpose(1, 0, 2).reshape(D, HL * HS)
        in_maps.append({
            "xT": np.ascontiguousarray(x[b].T),
            "wq": np.ascontiguousarray(wq_h),
            "wk": np.ascontiguousarray(wk_h),
            "wv": np.ascontiguousarray(wv_h),
            "wo": np.ascontiguousarray(
                W_out[hg * HL * HS:(hg + 1) * HL * HS, :]).astype(ml_dtypes.bfloat16),
            "mask": mask,
        })
    return in_maps


_NC_CACHE = {}


def get_nc():
    if "nc" not in _NC_CACHE:
        _NC_CACHE["nc"] = build()
    return _NC_CACHE["nc"]


def kernel(x, W_qkv, W_out):
    nc = get_nc()
    in_maps = make_in_maps(x, W_qkv, W_out)
    res = run_bass_kernel_spmd(nc, in_maps, list(range(8)))
    out = np.empty((B, T, D), dtype=np.float32)
    for b in range(B):
        out[b] = res.results[2 * b]["out"]
    return out


# revision 13
# speedup vs baseline: 1.6879x; 1.6879x over previous
"""Trainium2 Bass kernel for nn_MultiHeadAttention_50861002719805.

Full inputs in, full output out. Sharding: 8 cores = 4 batches x 2 head-groups
(tensor-parallel over heads, data-parallel over batch). Each core computes
attention for its batch + 8 heads. The pair {2b, 2b+1} exchanges normalized
per-head outputs (bf16 O^T, 512KB per pair, chunked AllGathers overlapped with
compute), then each core projects ALL 16 heads into its own half of the output
columns (host slices W_out columns per core), so no AllReduce is needed.

Per-core algorithm (all in transposed "head-dim on partitions" layout):
  Q^T = (Wq/32)^T x^T        [64,T] per head   (C**-0.5 folded into Wq)
  K^T = Wk^T x^T             [64,T]
  V   = x Wv                 [T,64]
  S^T[s,t] = K^T[:,s].Q^T[:,t]  computed per [128s x 512t] tile, fp32r,
  two heads row-packed on PE row groups 0/64.
  E = exp(S) (no max-shift needed: |S|<~1.5), masked entries := 1.0
    (faithful to the reference bug: masked scores = 1e-9, exp(1e-9)==1.0f)
  Fully-masked s-tiles (s0 >= t0+512) are skipped; their contribution is the
  rank-1 suffix sum_{s>=t0+512} v_aug[s] (incl. Z count), added as a K=1 MM.
  O^T_aug[65,512] = sum_s v_aug[s,:].E[s,t], v_aug = [1 | v] so row 0 = Z.
  O^T_norm = O^T * (1/Z) broadcast, bf16, DMA'd to DRAM, pair-AllGathered.
  out[t, my 512 cols] = sum_jj O_all[:,jj,t]^T @ W_out[jj rows, my cols].
"""
import numpy as np
import ml_dtypes

import concourse.bacc as bacc
import concourse.mybir as mybir
import concourse.tile as tile
from concourse.bass_utils import run_bass_kernel_spmd

F32 = mybir.dt.float32
F32R = mybir.dt.float32r
BF16 = mybir.dt.bfloat16
U8 = mybir.dt.uint8

B, T, D = 4, 2048, 1024
H, HS = 16, 64          # global heads, head size
HL = 8                  # heads per core
TCH, SCH = 512, 128     # t-chunk (psum free dim), s-chunk (partition tile)
NTC, NSC = T // TCH, T // SCH   # 4, 16
NDC = D // 128          # 8 contraction chunks
NP = 4                  # head pairs per core
ADD = mybir.AluOpType.add
MULT = mybir.AluOpType.mult
BYPASS = mybir.AluOpType.bypass
GROUPS = [[0, 1], [2, 3], [4, 5], [6, 7]]


def build(reps=1, collective=True, normalize=True):
    nc = bacc.Bacc("TRN2", target_bir_lowering=False, debug=False, num_devices=8)

    xT = nc.declare_dram_parameter("xT", [D, T], F32R, isOutput=False)
    wq = nc.declare_dram_parameter("wq", [D, HL * HS], F32R, isOutput=False)
    wk = nc.declare_dram_parameter("wk", [D, HL * HS], F32R, isOutput=False)
    wv = nc.declare_dram_parameter("wv", [D, HL * HS], F32R, isOutput=False)
    wo = nc.declare_dram_parameter("wo", [D, TCH], BF16, isOutput=False)
    mask = nc.declare_dram_parameter("mask", [4, SCH, TCH], U8, isOutput=False)
    out = nc.declare_dram_parameter("out", [T, TCH], F32, isOutput=True)

    with tile.TileContext(nc) as tc:
      for rep in range(reps):
        with (
            tc.tile_pool(name=f"const{rep}", bufs=1) as cpool,
            tc.tile_pool(name=f"wpool{rep}", bufs=1) as wpool,
            tc.tile_pool(name=f"vstp{rep}", bufs=1) as vstp,
            tc.tile_pool(name=f"small{rep}", bufs=3) as sp,
            tc.tile_pool(name=f"dram{rep}", bufs=1, space="DRAM") as dp,
        ):
            o_my = [dp.tile([128, T], BF16, name=f"omy{rep}_{j}") for j in range(NP)]
            o_all = [dp.tile([2, 128, T], BF16, name=f"oall{rep}_{j}") for j in range(NP)]
            # ---- constants ----
            ones_col_bf = cpool.tile([128, 1], BF16)        # chunk-sum lhsT
            ones_t_bf = cpool.tile([128, TCH], BF16)        # masked-fill data
            ones_f = cpool.tile([1, TCH], F32)
            ones_r = cpool.tile([1, TCH], F32R)             # rank-1 rhs
            nc.vector.memset(ones_col_bf[:], 1.0)
            nc.vector.memset(ones_t_bf[:], 1.0)
            nc.vector.memset(ones_f[:], 1.0)
            nc.vector.tensor_copy(ones_r[:], ones_f[:])

            mask_sb = cpool.tile([SCH, 4, TCH], U8)
            for k in range(4):
                nc.sync.dma_start(mask_sb[:, k, :], mask[k, :, :])

            # ---- weights ----
            wq_sb = wpool.tile([128, NDC, HL * HS], F32R)
            wk_sb = wpool.tile([128, NDC, HL * HS], F32R)
            wv_sb = wpool.tile([128, NDC, HL * HS], F32R)
            wo_sb = wpool.tile([128, NDC, TCH], BF16)
            for dc in range(NDC):
                nc.sync.dma_start(wq_sb[:, dc, :], wq[dc * 128:(dc + 1) * 128, :])
                nc.sync.dma_start(wk_sb[:, dc, :], wk[dc * 128:(dc + 1) * 128, :])
                nc.sync.dma_start(wv_sb[:, dc, :], wv[dc * 128:(dc + 1) * 128, :])
                nc.sync.dma_start(wo_sb[:, dc, :], wo[dc * 128:(dc + 1) * 128, :])

            # ---- V phase: V_st[p, sc, h, 0]=1 (Z col), cols 1:65 = v ----
            V_st = vstp.tile([SCH, NSC, HL, HS + 1], BF16)
            nc.vector.memset(V_st[:], 1.0)

            with (
                tc.tile_pool(name=f"xp{rep}", bufs=2) as xp,
                tc.tile_pool(name=f"qkt{rep}", bufs=2) as qkt,
                tc.tile_pool(name=f"ep{rep}", bufs=2) as ep,
            ):
                with tc.tile_pool(name=f"vps{rep}", bufs=1, space="PSUM") as vps:
                    for sub in range(2):
                        pv = [vps.tile([SCH, HL, HS], F32, tag=f"v{i}",
                                       name=f"pv{rep}_{sub}_{i}") for i in range(8)]
                        for dc in range(NDC):
                            xt = xp.tile([128, T], F32R, tag="xqk",
                                         name=f"xtv{rep}_{sub}_{dc}")
                            nc.sync.dma_start(
                                xt[:, 0:T // 2], xT[dc * 128:(dc + 1) * 128,
                                                    sub * 1024:(sub + 1) * 1024])
                            for i in range(8):
                                nc.tensor.matmul(
                                    pv[i][:], xt[:, i * 128:(i + 1) * 128],
                                    wv_sb[:, dc, :],
                                    start=(dc == 0), stop=(dc == NDC - 1))
                        for i in range(8):
                            sc = sub * 8 + i
                            nc.vector.tensor_copy(V_st[:, sc, :, 1:HS + 1], pv[i][:])

                # ---- suffix sums incl. masked-count (col 0 of each head) ----
                HWID = 4 * (HS + 1)  # 260
                vsuf_r = cpool.tile([1, 3, 2, HWID], F32R)
                with tc.tile_pool(name=f"sfps{rep}", bufs=1, space="PSUM") as sfps:
                    for tcb in range(3):
                        for half in range(2):
                            psf = sfps.tile([1, HWID], F32, tag=f"sf{tcb}{half}")
                            lo = 4 * (tcb + 1)
                            for c in range(lo, NSC):
                                nc.tensor.matmul(
                                    psf[:], ones_col_bf[:],
                                    V_st[:, c, half * 4:(half + 1) * 4, :],
                                    start=(c == lo), stop=(c == NSC - 1))
                            nc.vector.tensor_copy(vsuf_r[0:1, tcb, half, :], psf[:])

                for j in range(NP):
                    # -- QK phase for pair j --
                    QT = qkt.tile([128, NTC, TCH], F32R, tag="qt")
                    KT = qkt.tile([128, NTC, TCH], F32R, tag="kt")
                    with tc.tile_pool(name=f"qkps{rep}_{j}", bufs=1,
                                      space="PSUM") as qkps:
                        pq = [qkps.tile([128, TCH], F32, tag=f"q{i}",
                                        name=f"pq{rep}_{j}_{i}") for i in range(NTC)]
                        pk = [qkps.tile([128, TCH], F32, tag=f"k{i}",
                                        name=f"pk{rep}_{j}_{i}") for i in range(NTC)]
                        for dc in range(NDC):
                            xt = xp.tile([128, T], F32R, tag="xqk")
                            nc.sync.dma_start(xt[:], xT[dc * 128:(dc + 1) * 128, :])
                            for tcb in range(NTC):
                                nc.tensor.matmul(
                                    pq[tcb][:],
                                    wq_sb[:, dc, j * 128:(j + 1) * 128],
                                    xt[:, tcb * TCH:(tcb + 1) * TCH],
                                    start=(dc == 0), stop=(dc == NDC - 1))
                                nc.tensor.matmul(
                                    pk[tcb][:],
                                    wk_sb[:, dc, j * 128:(j + 1) * 128],
                                    xt[:, tcb * TCH:(tcb + 1) * TCH],
                                    start=(dc == 0), stop=(dc == NDC - 1))
                        for tcb in range(NTC):
                            nc.vector.tensor_copy(QT[:, tcb, :], pq[tcb][:])
                            nc.vector.tensor_copy(KT[:, tcb, :], pk[tcb][:])

                    # -- attention for heads (2j, 2j+1) --
                    with (
                        tc.tile_pool(name=f"sps{rep}_{j}", bufs=4,
                                     space="PSUM") as spsum,
                        tc.tile_pool(name=f"ops{rep}_{j}", bufs=4,
                                     space="PSUM") as opsum,
                    ):
                        for tcb in range(NTC):
                            nv = 4 * (tcb + 1)   # valid s-chunks
                            E = [ep.tile([SCH, NSC, TCH], BF16, tag="E",
                                         name=f"E{rep}_{j}_{tcb}_{ee}")
                                 for ee in range(2)]
                            po = [opsum.tile([HS + 1, TCH], F32, tag="po",
                                             name=f"po{rep}_{j}_{tcb}_{ee}")
                                  for ee in range(2)]
                            for c in range(nv):
                                ps = [None, None]
                                for e in range(2):
                                    ps[e] = spsum.tile(
                                        [SCH, TCH], F32, tag="ps",
                                        name=f"ps{rep}_{j}_{tcb}_{c}_{e}")
                                    nc.tensor.matmul(
                                        ps[e][:],
                                        KT[64 * e:64 * e + 64, c // 4,
                                           (c % 4) * SCH:(c % 4 + 1) * SCH],
                                        QT[64 * e:64 * e + 64, tcb, :],
                                        start=True, stop=True)
                                for e in range(2):
                                    nc.scalar.activation(
                                        E[e][:, c, :], ps[e][:],
                                        mybir.ActivationFunctionType.Exp)
                                    if c >= 4 * tcb:
                                        nc.vector.copy_predicated(
                                            E[e][:, c, :],
                                            mask_sb[:, c - 4 * tcb, :],
                                            ones_t_bf[:])
                                for e in range(2):
                                    h = 2 * j + e
                                    nc.tensor.matmul(
                                        po[e][:],
                                        V_st[:, c, h, :],
                                        E[e][:, c, :],
                                        start=(c == 0),
                                        stop=(c == nv - 1 and tcb == 3),
                                        skip_group_check=True)
                            if tcb < 3:
                                for e in range(2):
                                    h = 2 * j + e
                                    nc.tensor.matmul(
                                        po[e][:],
                                        vsuf_r[0:1, tcb, j // 2,
                                               (h % 4) * (HS + 1):
                                               (h % 4 + 1) * (HS + 1)],
                                        ones_r[:],
                                        start=False, stop=True,
                                        skip_group_check=True)
                            # -- normalize: O^T / Z (Z is psum row 0) --
                            for e in range(2):
                                og = sp.tile([HS + 1, TCH], BF16, tag="og")
                                if normalize:
                                    rp0 = sp.tile([1, TCH], F32, tag="rp0")
                                    nc.vector.reciprocal(rp0[:], po[e][0:1, :])
                                    rbc = sp.tile([HS + 1, TCH], F32, tag="rbc")
                                    nc.gpsimd.partition_broadcast(
                                        rbc[:], rp0[:], channels=HS + 1)
                                    nc.vector.tensor_tensor(
                                        og[:], po[e][:], rbc[:], MULT)
                                else:
                                    nc.vector.tensor_copy(og[:], po[e][:])
                                nc.sync.dma_start(
                                    o_my[j][64 * e:64 * e + 64,
                                            tcb * TCH:(tcb + 1) * TCH],
                                    og[1:HS + 1, :])

                    # -- exchange this pair's O^T with the partner core --
                    if collective:
                        nc.gpsimd.collective_compute(
                            "AllGather", BYPASS,
                            replica_groups=GROUPS,
                            ins=[o_my[j][:]],
                            outs=[o_all[j][:]],
                        )

            # ---- output projection: all 16 heads x my 512 out columns ----
            with (
                tc.tile_pool(name=f"projp{rep}", bufs=1) as projp,
                tc.tile_pool(name=f"outp{rep}", bufs=3) as outp,
                tc.tile_pool(name=f"pps{rep}", bufs=4, space="PSUM") as pps,
            ):
                O_sb = projp.tile([128, 2, NP, T], BF16)
                for j in range(NP):
                    for g in range(2):
                        src = o_all[j][g, :, :] if collective else o_my[j][:]
                        nc.sync.dma_start(O_sb[:, g, j, :], src)
                for tt in range(T // 128):
                    pp = pps.tile([128, TCH], F32, tag="pp", name=f"pp{rep}_{tt}")
                    for jj in range(NDC):
                        g, j = jj // 4, jj % 4
                        nc.tensor.matmul(
                            pp[:],
                            O_sb[:, g, j, tt * 128:(tt + 1) * 128],
                            wo_sb[:, jj, :],
                            start=(jj == 0), stop=(jj == NDC - 1))
                    ob = outp.tile([128, TCH], F32, tag="ob", name=f"ob{rep}_{tt}")
                    nc.vector.tensor_copy(ob[:], pp[:])
                    nc.sync.dma_start(out[tt * 128:(tt + 1) * 128, :], ob[:])

    nc.compile()
    return nc


def make_mask():
    # mask[k][p, f] = 1 where masked: s > t  <=>  p + 128k > f
    p = np.arange(SCH)[:, None]
    f = np.arange(TCH)[None, :]
    return np.stack([(p + 128 * k > f) for k in range(4)]).astype(np.uint8)


def make_in_maps(x, W_qkv, W_out):
    x = np.asarray(x, dtype=np.float32)
    W_qkv = np.asarray(W_qkv, dtype=np.float32)
    W_out = np.asarray(W_out, dtype=np.float32)
    mask = make_mask()
    in_maps = []
    for c in range(8):
        b, hg = c // 2, c % 2
        heads = slice(hg * HL, (hg + 1) * HL)
        # [h, d, f] -> [d, h, f] -> [d, h*f]
        wq_h = W_qkv[heads, :, 0:HS].transpose(1, 0, 2).reshape(D, HL * HS) * (1.0 / 32.0)
        wk_h = W_qkv[heads, :, HS:2 * HS].transpose(1, 0, 2).reshape(D, HL * HS)
        wv_h = W_qkv[heads, :, 2 * HS:3 * HS].transpose(1, 0, 2).reshape(D, HL * HS)
        in_maps.append({
            "xT": np.ascontiguousarray(x[b].T),
            "wq": np.ascontiguousarray(wq_h),
            "wk": np.ascontiguousarray(wk_h),
            "wv": np.ascontiguousarray(wv_h),
            "wo": np.ascontiguousarray(
                W_out[:, hg * TCH:(hg + 1) * TCH]).astype(ml_dtypes.bfloat16),
            "mask": mask,
        })
    return in_maps


_NC_CACHE = {}


def get_nc():
    if "nc" not in _NC_CACHE:
        _NC_CACHE["nc"] = build()
    return _NC_CACHE["nc"]


def kernel(x, W_qkv, W_out):
    nc = get_nc()
    in_maps = make_in_maps(x, W_qkv, W_out)
    res = run_bass_kernel_spmd(nc, in_maps, list(range(8)))
    out = np.empty((B, T, D), dtype=np.float32)
    for b in range(B):
        out[b, :, 0:TCH] = res.results[2 * b]["out"]
        out[b, :, TCH:D] = res.results[2 * b + 1]["out"]
    return out
